# revision 20
# baseline (speedup 1.0000x reference)
"""Local-strided block-sparse paged attention (decode) on 8 Trainium2 cores.

Strategy (v2 — 16-block-granular stream packing):
- Work unit = (sequence b, kv-head kv): the 4 q-heads sharing a kv head
  attend overlapping block sets, so each unit loads the UNION of its 4
  heads' CSR rows once; per-head 0/1 masks applied after exp restore row
  membership + causality. Panels are bf16 (tolerance 2e-2, bf16 ~3e-3).
- 64 units sorted by size, rank r = units[8r:8r+8] (consecutive-sorted
  is optimal for sum-of-rank-maxima); core c takes the c-th unit of each
  rank. Every core pads each unit to its rank max IN BLOCKS (16 tokens),
  not chunks, then concatenates the 8 ranks into ONE token stream cut
  into 128-token chunks — a chunk can straddle two ranks. This replaces
  v1's per-unit 128-rounding + chunk-matched slots (74 chunks/core) with
  block-granular matching (70 chunks/core), ~6% less HBM traffic.
- Per chunk the program computes scores for BOTH possibly-present ranks
  into 8 score columns (even-parity rank -> cols 0-3, odd -> 4-7); both
  matmuls share the chunk's stationary K panel. Garbage halves are
  masked to zero (masks are host-built), which also keeps every PSUM
  element written (exp of unwritten PSUM would poison with NaN).
- DMA plan (all measured): per-core HBM bandwidth is the roofline
  (~325 GB/s with 8 cores streaming, chip-limited), so the layout
  minimizes bytes and keeps both HWDGE rings busy. The K panel
  streams in 3 pieces on the SP ring (preceded by q and the fp8
  mask panel — masks are tiny and arrive first so mask-muls never
  gate on V); the V+ones panel streams in 4 pieces on the Act ring.
  Piecewise transfers matter because a mega-DMA's completion
  semaphore fires only at the end, gating all dependent compute on
  the full stream; a few large pieces let QK(g)/PV(g) track the
  stream at ~0.1us extra ring time per piece. The output store uses
  the SWDGE (gpsimd) path: on either HWDGE ring its compute-tail
  wait would FIFO-block the next iteration's stream triggers.
- Compute is software-pipelined one group ahead (QK(g+1) emitted
  before PV(g)) so the in-order PE never stalls on the exp->mask
  chain of the current group.
- PV accumulates per rank into a [4,129] PSUM tile across the rank's
  chunk segments; the V panels carry a ones-column so the softmax
  denominator falls out of the same accumulation; per-rank normalize
  into one output tile, single store.
"""
import math
import numpy as np
import ml_dtypes

NCORES = 8
NRANKS = 8
CG = 8          # chunks per score/exp/mask group
KPIECES = 3     # K stream pieces (SP ring)
VPIECES = 4     # V stream pieces (Act ring)
_PROG_CACHE: dict = {}


def _resolve_rows(layout_crow, layout_col, pbid, H, J=64):
    """Mirror the reference CSR row resolution (first-J trim + idx clip)."""
    W = layout_col.shape[1]
    rows = []
    for h in range(H):
        s = int(layout_crow[h, pbid])
        e = int(layout_crow[h, pbid + 1])
        n = min(max(e - s, 0), J)
        idx = np.clip(np.arange(s, s + n), 0, W - 1)
        rows.append(layout_col[h, idx].tolist())
    return rows


def _schedule(sig):
    """Static per-core schedule shared by host packing and program build.

    Returns (spans, NCH, groups): spans[r] is the token span of rank r
    (rank 7 extended over the final chunk pad), groups is a list of
    (first_chunk, n_chunks) pairs."""
    rank_blocks = list(sig[:-2])
    off = np.concatenate([[0], np.cumsum(rank_blocks)])
    TB = int(off[-1])
    NCH = -(-TB * 16 // 128)
    spans = []
    for r in range(NRANKS):
        t0, t1 = int(off[r]) * 16, int(off[r + 1]) * 16
        if r == NRANKS - 1:
            t1 = NCH * 128
        spans.append((t0, t1))
    groups = []
    c = 0
    while c < NCH:
        cg = min(CG, NCH - c)
        groups.append((c, cg))
        c += cg
    return spans, NCH, groups


def _prepare(q, k_cache, v_cache, block_tables, context_lens, layout_crow, layout_col):
    B, H, D = q.shape
    KVH = k_cache.shape[1]
    BLK = v_cache.shape[3]
    G = H // KVH
    q_pid = context_lens.astype(np.int64) - 1
    pbid = q_pid // BLK

    bf16 = ml_dtypes.bfloat16

    # ---- build units: (b, kv) -> union block list + per-head membership ----
    units = []
    for b in range(B):
        rows_all = _resolve_rows(layout_crow, layout_col, int(pbid[b]), H)
        for kv in range(KVH):
            heads = [kv * G + j for j in range(G)]
            cnts = []
            for h in heads:
                c = {}
                for kb in rows_all[h]:
                    c[kb] = c.get(kb, 0) + 1
                cnts.append(c)
            mult = {}
            for c in cnts:
                for kb, n in c.items():
                    mult[kb] = max(mult.get(kb, 0), n)
            ulist = []
            copyidx = []
            for kb in sorted(mult):
                for i in range(mult[kb]):
                    ulist.append(kb)
                    copyidx.append(i)
            units.append(dict(b=b, kv=kv, heads=heads, ulist=ulist,
                              copyidx=copyidx, cnts=cnts, U=len(ulist)))

    # ---- deal: sort desc, rank r = 8 consecutive units, one per core ----
    assert len(units) == NCORES * NRANKS
    order = sorted(range(len(units)), key=lambda i: -units[i]["U"])
    rank_blocks = []
    assign = [[None] * NRANKS for _ in range(NCORES)]
    for r in range(NRANKS):
        grp = order[r * NCORES:(r + 1) * NCORES]
        rank_blocks.append(max(units[i]["U"] for i in grp))
        for c, i in enumerate(grp):
            assign[c][r] = i
    sig = tuple(rank_blocks) + (G, D)
    spans, NCH, groups = _schedule(sig)
    NT = NCH * 128

    # ---- build per-core panels ----
    in_maps = []
    for c in range(NCORES):
        kd = np.zeros((D, NT), bf16)
        vtok = np.zeros((NT, D), np.float32)
        mtok = np.zeros((NT, 2 * G), np.float32)
        qq = np.zeros((D, NRANKS * G), bf16)
        for r in range(NRANKS):
            u = units[assign[c][r]]
            b, kv, U = u["b"], u["kv"], u["U"]
            t0 = spans[r][0]
            phys = block_tables[b, np.asarray(u["ulist"], np.int64)]
            kb = k_cache[phys, kv]          # [U, D//X, BLK, X]
            kd[:, t0:t0 + U * BLK] = (
                kb.transpose(1, 3, 0, 2).reshape(D, U * BLK).astype(bf16))
            vtok[t0:t0 + U * BLK] = (
                v_cache[phys, kv].transpose(0, 2, 1).reshape(U * BLK, D))
            ul = np.asarray(u["ulist"], np.int64)
            ci = np.asarray(u["copyidx"], np.int64)
            pos = (ul[:, None] * BLK + np.arange(BLK)[None, :]).reshape(-1)
            par = (r % 2) * G
            for j in range(G):
                cnt = u["cnts"][j]
                member = np.asarray(
                    [ci[i] < cnt.get(int(ul[i]), 0) for i in range(U)], bool)
                ok = np.repeat(member, BLK) & (pos <= int(q_pid[b]))
                mtok[t0:t0 + U * BLK, par + j] = ok.astype(np.float32)
            qq[:, r * G:(r + 1) * G] = q[b, u["heads"]].T.astype(bf16)

        # V+ones panel [128, NCH*129]; masks ride a separate fp8 tensor
        # (0/1 is exact in fp8) streamed first so mask-muls never gate
        # on the V stream.
        va = vtok.reshape(NCH, 128, D)
        va = np.concatenate([va, np.ones((NCH, 128, 1), np.float32)], 2)
        vgp = va.transpose(1, 0, 2).reshape(128, NCH * 129)
        mgp = mtok.reshape(NCH, 128, 2 * G).transpose(1, 0, 2).reshape(
            128, NCH * 2 * G)
        m = {"kg": kd,
             "vg": vgp.astype(bf16),
             "mg": mgp.astype(ml_dtypes.float8_e4m3),
             "qq": qq}
        in_maps.append(m)
    return in_maps, assign, units, sig, NRANKS


def _build_program(sig, repeat=1, loop=0):
    import concourse.bacc as bacc
    import concourse.mybir as mybir
    from concourse.tile import TileContext

    G, D = sig[-2], sig[-1]
    spans, NCH, groups = _schedule(sig)
    NT = NCH * 128
    W8 = 2 * G                      # score cols per chunk
    VW = NCH * 129                  # vg panel cols (V+ones only)
    f32 = mybir.dt.float32
    bf16 = mybir.dt.bfloat16
    SM = 1.0 / math.sqrt(D)

    # chunk -> (even-parity rank or fallback, odd-parity rank or fallback)
    chunk_ranks = []
    for c in range(NCH):
        lo, hi = c * 128, (c + 1) * 128
        pres = [r for r in range(NRANKS)
                if spans[r][0] < hi and spans[r][1] > lo]
        assert 1 <= len(pres) <= 2
        er = next((r for r in pres if r % 2 == 0), None)
        orr = next((r for r in pres if r % 2 == 1), None)
        if er is None:
            er = orr - 1 if orr > 0 else orr + 1
        if orr is None:
            orr = er + 1 if er < NRANKS - 1 else er - 1
        chunk_ranks.append((er, orr))
    rank_first_chunk = [spans[r][0] // 128 for r in range(NRANKS)]
    rank_last_chunk = [-(-spans[r][1] // 128) - 1 for r in range(NRANKS)]

    nc = bacc.Bacc("TRN2", target_bir_lowering=False)
    kg = nc.dram_tensor("kg", [D, NT], bf16, kind="ExternalInput")
    vg = nc.dram_tensor("vg", [128, VW], bf16, kind="ExternalInput")
    mg = nc.dram_tensor("mg", [128, NCH * W8], mybir.dt.float8e4,
                        kind="ExternalInput")
    qq = nc.dram_tensor("qq", [D, NRANKS * G], bf16, kind="ExternalInput")
    out = nc.dram_tensor("out", [G, NRANKS * D], f32, kind="ExternalOutput")


    with TileContext(nc) as tc:
        with (
            tc.tile_pool(name="kv", bufs=2) as kvp,
            tc.tile_pool(name="small", bufs=4) as sp,
            tc.tile_pool(name="ps_sc", bufs=4, space="PSUM") as pp_sc,
            tc.tile_pool(name="ps_ov", bufs=4, space="PSUM") as pp_ov,
            tc.tile_pool(name="persist", bufs=2) as cp,
        ):
            def _one_body():
                qt = cp.tile([D, NRANKS * G], bf16, tag="qt")
                nc.sync.dma_start(out=qt[:], in_=qq[:])
                kgt = kvp.tile([D, NT], bf16, tag="kg")
                vgt = kvp.tile([128, VW], bf16, tag="vg")
                mgt = kvp.tile([128, NCH * W8], mybir.dt.float8e4, tag="mg")
                nc.sync.dma_start(out=mgt[:], in_=mg[:])
                # Piecewise transfers: a single mega-DMA's completion
                # semaphore only fires at the end, gating ALL dependent
                # compute on the full stream; but each extra transfer
                # costs ~0.1us of ring time, so split into a few large
                # pieces only (piece boundaries at group boundaries).
                ng = len(groups)
                kcut = [round(i * ng / KPIECES) for i in range(KPIECES + 1)]
                vcut = [round(i * ng / VPIECES) for i in range(VPIECES + 1)]

                def _dma_k(a, b):
                    c0, c1 = groups[a][0], groups[b - 1][0] + groups[b - 1][1]
                    nc.sync.dma_start(
                        out=kgt[:, c0 * 128:c1 * 128],
                        in_=kg[:, c0 * 128:c1 * 128])

                def _dma_v(a, b):
                    o0 = groups[a][0] * 129
                    o1 = (groups[b - 1][0] + groups[b - 1][1]) * 129
                    # V rides the Act HWDGE ring, K the SP ring: the
                    # two streams share HBM bandwidth but pipeline
                    # per-piece, so QK(g) and PV(g) both track the
                    # stream instead of PV waiting out the full K
                    # stream first.
                    nc.scalar.dma_start(out=vgt[:, o0:o1], in_=vg[:, o0:o1])

                for i in range(KPIECES):
                    _dma_k(kcut[i], kcut[i + 1])
                for i in range(VPIECES):
                    _dma_v(vcut[i], vcut[i + 1])
                osb = cp.tile([G, NRANKS * D], f32, tag="osb")
                ov = [None] * NRANKS

                def _qk(gi):
                    g0, cg = groups[gi]
                    sc = pp_sc.tile([128, cg * W8], f32, tag="sc", name="sc")
                    for ci in range(cg):
                        c = g0 + ci
                        kt = kgt[:, c * 128:(c + 1) * 128]
                        er, orr = chunk_ranks[c]
                        nc.tensor.matmul(
                            sc[:, ci * W8:ci * W8 + G], kt,
                            qt[:, er * G:(er + 1) * G],
                            start=True, stop=True)
                        nc.tensor.matmul(
                            sc[:, ci * W8 + G:ci * W8 + W8], kt,
                            qt[:, orr * G:(orr + 1) * G],
                            start=True, stop=True)
                    pe = sp.tile([128, cg * W8], bf16, tag="pe", name="pe")
                    nc.scalar.activation(
                        pe[:], sc[:], mybir.ActivationFunctionType.Exp,
                        scale=SM)
                    p = sp.tile([128, cg * W8], bf16, tag="p", name="p")
                    mt = mgt[:, g0 * W8:(g0 + cg) * W8]
                    nc.vector.tensor_mul(p[:], pe[:], mt)
                    return p

                def _pv(gi, p):
                    g0, cg = groups[gi]
                    for ci in range(cg):
                        c = g0 + ci
                        vt = vgt[:, c * 129:(c + 1) * 129]
                        for r in sorted(set(chunk_ranks[c])):
                            t0, t1 = spans[r]
                            lo = max(t0, c * 128) - c * 128
                            hi = min(t1, (c + 1) * 128) - c * 128
                            if hi <= lo:
                                continue
                            par = (r % 2) * G
                            if ov[r] is None:
                                ov[r] = pp_ov.tile(
                                    [G, 129], f32, tag="ov", name=f"ov{r}")
                            # Full-chunk rows: base_partition must be
                            # 0/32/64, and rows outside the rank's span
                            # have p==0 in its parity columns (host
                            # masks), so they contribute exactly zero.
                            nc.tensor.matmul(
                                ov[r][:],
                                p[:, ci * W8 + par:ci * W8 + par + G],
                                vt[:, :],
                                start=(c == rank_first_chunk[r]),
                                stop=(c == rank_last_chunk[r]))
                            if c == rank_last_chunk[r]:
                                rec = sp.tile([G, 1], f32, tag="rec",
                                              name="rec")
                                nc.vector.reciprocal(rec[:], ov[r][:, 128:129])
                                nc.vector.tensor_scalar_mul(
                                    osb[:, r * D:(r + 1) * D],
                                    ov[r][:, 0:128], rec[:])

                # Software-pipeline: emit QK(g+1) before PV(g) so the
                # in-order PE never idles waiting on group g's exp/mask
                # chain — it streams the next group's scores meanwhile.
                pprev = _qk(0)
                for gi in range(1, len(groups)):
                    pcur = _qk(gi)
                    _pv(gi - 1, pprev)
                    pprev = pcur
                _pv(len(groups) - 1, pprev)
                # Output store rides the SWDGE (gpsimd) path: on either
                # HWDGE ring its compute-tail wait would FIFO-block the
                # next iteration's K/V stream triggers.
                nc.gpsimd.dma_start(out=out[:], in_=osb[:])

            if loop:
                with tc.For_i(0, loop, 1,
                              hint_engines=(mybir.EngineType.PE,
                                            mybir.EngineType.DVE,
                                            mybir.EngineType.Activation)):
                    for _rep in range(repeat):
                        _one_body()
            else:
                for _rep in range(repeat):
                    _one_body()
    nc.compile()
    return nc


def _get_program(sig, repeat=1, loop=0):
    key = (sig, repeat, loop)
    nc = _PROG_CACHE.get(key)
    if nc is None:
        nc = _build_program(sig, repeat, loop)
        _PROG_CACHE[key] = nc
    return nc


def kernel(q, k_cache, v_cache, block_tables, context_lens, layout_crow, layout_col):
    from concourse.bass_utils import run_bass_kernel_spmd

    q = np.asarray(q, np.float32)
    k_cache = np.asarray(k_cache, np.float32)
    v_cache = np.asarray(v_cache, np.float32)
    block_tables = np.asarray(block_tables, np.int64)
    context_lens = np.asarray(context_lens, np.int64)
    layout_crow = np.asarray(layout_crow, np.int64)
    layout_col = np.asarray(layout_col, np.int64)

    B, H, D = q.shape

    in_maps, assign, units, sig, nranks = _prepare(
        q, k_cache, v_cache, block_tables, context_lens, layout_crow, layout_col)

    nc = _get_program(sig)

    res = run_bass_kernel_spmd(nc, in_maps, core_ids=list(range(NCORES)))

    out = np.empty((B, H, D), np.float32)
    for c in range(NCORES):
        o = res.results[c]["out"]
        for r in range(nranks):
            u = units[assign[c][r]]
            out[u["b"], u["heads"]] = o[:, r * D:(r + 1) * D]
    return out
